# revision 6
# baseline (speedup 1.0000x reference)
"""Trainium2 Bass kernel for nn_AttentionMax (batched dot-product argmax one-hot).

corr[b, s] = <feat_query[b], feat_sub[b, s]>   (bz=4096, n_support=256, d=128)
out[b, s, 0] = one_hot(argmax_s corr[b])

Sharding: pure data parallel over the batch dim across 8 NeuronCores.
Per core: 512 batches = 4 blocks of 128 (partition dim = batch).
Per block: stream feat_sub in [128, SC, 128] chunks (contiguous 32KB per
partition per chunk), multiply by query (0-stride broadcast along s) on
VectorE, reduce over d, then an exact first-argmax one-hot via
reduce_max / is_equal*iota (fused) / reduce_min / is_equal.

DMA-produced tiles (q, iota) are first copied via VectorE so that each
compute instruction carries at most one cross-engine semaphore wait
(walrus rejects instructions with too many sync waits).
"""

import sys

if "/opt/trn_rl_repo" not in sys.path:
    sys.path.insert(0, "/opt/trn_rl_repo")

import numpy as np

import concourse.bass as bass
import concourse.mybir as mybir
from concourse import bacc, tile
from concourse.bass_utils import run_bass_kernel_spmd

N_CORES = 8
BZ = 4096
BZL = BZ // N_CORES  # 512 batches per core
NS = 256  # n_support
D = 128
P = 128  # batches per block (partition dim)
NBLK = BZL // P  # 4
SC = 64  # s-chunk per DMA/compute slot

F32 = mybir.dt.float32


def _build():
    nc = bacc.Bacc("TRN2", target_bir_lowering=False, debug=False)
    fq = nc.declare_dram_parameter("feat_query", [BZL, D], F32, isOutput=False)
    fs = nc.declare_dram_parameter("feat_sub", [BZL, NS, D], F32, isOutput=False)
    iota = nc.declare_dram_parameter("iota", [P, NS], F32, isOutput=False)
    out = nc.declare_dram_parameter("out", [BZL, NS], F32, isOutput=True)

    with tile.TileContext(nc) as tc:
        with (
            tc.tile_pool(name="sub", bufs=3) as sub_pool,
            tc.tile_pool(name="prod", bufs=2) as prod_pool,
            tc.tile_pool(name="qp", bufs=NBLK) as q_pool,
            tc.tile_pool(name="cp", bufs=NBLK) as c_pool,
            tc.tile_pool(name="const", bufs=1) as const_pool,
        ):
            iota_d = const_pool.tile([P, NS], F32)
            nc.scalar.dma_start(out=iota_d[:], in_=iota[:, :])
            iota_v = const_pool.tile([P, NS], F32)
            nc.vector.tensor_copy(iota_v[:], iota_d[:])

            for blk in range(NBLK):
                b0 = blk * P
                q_d = q_pool.tile([P, D], F32)
                nc.scalar.dma_start(out=q_d[:], in_=fq[b0 : b0 + P, :])
                q_v = q_pool.tile([P, D], F32)
                nc.vector.tensor_copy(q_v[:], q_d[:])
                corr = c_pool.tile([P, NS], F32)

                for ci in range(NS // SC):
                    sub_tile = sub_pool.tile([P, SC, D], F32)
                    nc.sync.dma_start(
                        out=sub_tile[:],
                        in_=fs[b0 : b0 + P, ci * SC : (ci + 1) * SC, :],
                    )
                    prod = prod_pool.tile([P, SC, D], F32)
                    q_b = q_v[:, :].unsqueeze(1).broadcast_to([P, SC, D])
                    nc.vector.tensor_tensor(
                        out=prod[:], in0=sub_tile[:], in1=q_b, op=mybir.AluOpType.mult
                    )
                    nc.vector.reduce_sum(
                        out=corr[:, ci * SC : (ci + 1) * SC],
                        in_=prod[:],
                        axis=mybir.AxisListType.X,
                    )

                rmax = c_pool.tile([P, 1], F32)
                nc.vector.reduce_max(out=rmax[:], in_=corr[:], axis=mybir.AxisListType.X)
                # masked = (corr == rmax) * (iota - 1024): matches -> idx-1024 (<0), else 0
                masked = c_pool.tile([P, NS], F32)
                nc.vector.scalar_tensor_tensor(
                    out=masked[:], in0=corr[:], scalar=rmax[:], in1=iota_v[:],
                    op0=mybir.AluOpType.is_equal, op1=mybir.AluOpType.mult,
                )
                rmin = c_pool.tile([P, 1], F32)
                nc.vector.tensor_reduce(
                    out=rmin[:], in_=masked[:], axis=mybir.AxisListType.X,
                    op=mybir.AluOpType.min,
                )
                onehot = c_pool.tile([P, NS], F32)
                nc.vector.tensor_scalar(
                    out=onehot[:], in0=iota_v[:], scalar1=rmin[:], scalar2=None,
                    op0=mybir.AluOpType.is_equal,
                )
                nc.scalar.dma_start(out=out[b0 : b0 + P, :], in_=onehot[:])

    nc.compile()
    return nc


_CACHE = {}


def _get_nc():
    if "nc" not in _CACHE:
        _CACHE["nc"] = _build()
    return _CACHE["nc"]


def _in_maps(feat_query, feat_sub):
    feat_query = np.ascontiguousarray(np.asarray(feat_query), dtype=np.float32)
    feat_sub = np.ascontiguousarray(np.asarray(feat_sub), dtype=np.float32)
    assert feat_query.shape == (BZ, D), feat_query.shape
    assert feat_sub.shape == (BZ, NS, D), feat_sub.shape
    iota_np = np.tile(np.arange(NS, dtype=np.float32) - 1024.0, (P, 1))
    maps = []
    for i in range(N_CORES):
        sl = slice(i * BZL, (i + 1) * BZL)
        maps.append(
            {
                "feat_query": feat_query[sl],
                "feat_sub": feat_sub[sl],
                "iota": iota_np,
            }
        )
    return maps


def _assemble(results):
    outs = [results[i]["out"] for i in range(N_CORES)]
    return np.concatenate(outs, axis=0).reshape(BZ, NS, 1).astype(np.float32)


def run(feat_query, feat_sub, trace=False):
    """Run on 8 NeuronCores; returns (output, BassKernelResults)."""
    nc = _get_nc()
    res = run_bass_kernel_spmd(
        nc, _in_maps(feat_query, feat_sub), list(range(N_CORES)), trace=trace
    )
    return _assemble(res.results), res


def kernel(feat_query, feat_sub):
    out, _ = run(feat_query, feat_sub, trace=False)
    return out
